# revision 1
# baseline (speedup 1.0000x reference)
"""Sliding-window GQA attention block (RoPE + QKV proj + SDPA + O proj) on 8
Trainium2 NeuronCores, head-sharded (1 kv-head group = 8 q-heads per core).

Contract: kernel(**inputs) takes the FULL unsharded inputs from
setup_inputs() and returns the FULL [1, 2048, 2880] output.

Per-core plan (core c owns q-heads [8c, 8c+8), kv-head c):
  - host passes x.T (padded, with a ones-row to fold biases into the matmul),
    per-core transposed weight slices, RoPE cos/sin tables (head-dim order
    permuted so the rotate-half partner is the adjacent partition, making the
    RoPE "rotate" a within-quadrant stream_shuffle), and additive mask tiles.
  - QKV projections as fp32r matmuls (stream 512-wide), RoPE in the PSUM
    epilogue, producing qT/kT in [head_dim, seq] layout (fp32r) and v in
    natural [seq, head_dim] layout (bf16, via PE transposes) with an
    appended all-ones block so the attention-value matmul also produces the
    softmax denominators (replicated across 64 partitions).
  - Attention in S^T layout: per key-tile j, scores.T [128 keys, 256 queries]
    (one fp32r matmul), additive sliding-window mask, exp (scale=1/8 folded),
    then bf16 AV matmuls accumulate out.T + denominators per query tile.
    Normalization = one reciprocal + one multiply per (head, query-tile).
  - O projection: fp32r matmuls over the 4 attn-out partition tiles,
    streaming wo.T; per-core partial [2048, 2880] returned to host.
  - host sums the 8 partials and adds wo_b.
"""
import sys

sys.path.insert(0, "/opt/trn_rl_repo")

import numpy as np

import concourse.bass as bass  # noqa: F401  (import keeps bass registered)
import concourse.tile as tile
from concourse import bacc, mybir
from concourse.bass_utils import run_bass_kernel_spmd

B, S, D = 1, 2048, 2880
H, KVH, HD = 64, 8, 64
WINDOW = 128
N_CORES = 8
DP = 2944  # padded contraction dim: 23 * 128 (2880 data + 1 ones row + pad)
KT = DP // 128  # 23 contraction tiles
NQT = S // 128  # 16 seq tiles
OCH = 480  # O-proj free chunk (6 * 480 = 2880)

F32R = mybir.dt.float32r
F32 = mybir.dt.float32
BF16 = mybir.dt.bfloat16

# head-dim permutation: pairs (t, t+32) adjacent -> rotate-half partner is
# the neighbouring partition (stream_shuffle mask i^1 within quadrants)
PERM = np.empty(HD, dtype=np.int64)
PERM[0::2] = np.arange(32)
PERM[1::2] = np.arange(32) + 32

_COMPILED = None


def _build(debug=False):
    nc = bacc.Bacc("TRN2", target_bir_lowering=False, debug=False)

    xT_d = nc.dram_tensor("xT", [DP, S], F32R, kind="ExternalInput").ap()
    wq_d = nc.dram_tensor("wq", [DP, 512], F32R, kind="ExternalInput").ap()
    wkv_d = nc.dram_tensor("wkv", [DP, 128], F32R, kind="ExternalInput").ap()
    wo_d = nc.dram_tensor("wo", [512, D], F32R, kind="ExternalInput").ap()
    cos_d = nc.dram_tensor("cosT", [128, S], F32, kind="ExternalInput").ap()
    sin_d = nc.dram_tensor("sinTs", [128, S], F32, kind="ExternalInput").ap()
    ma0_d = nc.dram_tensor("ma0", [128, 256], F32, kind="ExternalInput").ap()
    ma1_d = nc.dram_tensor("ma1", [128, 128], F32, kind="ExternalInput").ap()
    id_d = nc.dram_tensor("id64", [64, 64], BF16, kind="ExternalInput").ap()
    out_d = nc.dram_tensor("partial", [S, D], F32, kind="ExternalOutput").ap()
    if debug:
        dbg_qT_d = nc.dram_tensor("dbg_qT", [128, S], F32, kind="ExternalOutput").ap()
        dbg_kT_d = nc.dram_tensor("dbg_kT", [128, S], F32, kind="ExternalOutput").ap()
        dbg_vx_d = nc.dram_tensor("dbg_vx", [128, 128], F32, kind="ExternalOutput").ap()
        dbg_ao_d = nc.dram_tensor("dbg_ao", [128, S], F32, kind="ExternalOutput").ap()

    Exp = mybir.ActivationFunctionType.Exp
    SHUF_MASK = [i ^ 1 for i in range(32)]

    with tile.TileContext(nc) as tc:
        with (
            tc.tile_pool(name="constp", bufs=1) as constp,
            tc.tile_pool(name="qkvp", bufs=1) as qkvp,
            tc.tile_pool(name="vextp", bufs=1) as vextp,
            tc.tile_pool(name="workp", bufs=3) as workp,
        ):
            cos_t = constp.tile([128, S], F32)
            sin_t = constp.tile([128, S], F32)
            ma0_t = constp.tile([128, 256], F32)
            ma1_t = constp.tile([128, 128], F32)
            id_t = constp.tile([64, 64], BF16)
            nc.sync.dma_start(cos_t[:], cos_d[:])
            nc.sync.dma_start(sin_t[:], sin_d[:])
            nc.sync.dma_start(ma0_t[:], ma0_d[:])
            nc.sync.dma_start(ma1_t[:], ma1_d[:])
            nc.sync.dma_start(id_t[:], id_d[:])

            # qTm[mt] holds heads (2mt, 2mt+1); kT2 holds kT duplicated in both
            # partition halves so scores lhsT/rhs base partitions match.
            qTm = [qkvp.tile([128, S], F32R, name=f"qTm{t}") for t in range(4)]
            kT2 = qkvp.tile([128, S], F32R, name="kT2")
            vT = qkvp.tile([64, S], BF16, name="vT")
            v_ext = [vextp.tile([128, 128], BF16, name=f"vx{i}") for i in range(NQT)]

            # ---------------- Phase 1: QKV projections + RoPE ----------------
            with (
                tc.tile_pool(name="wpool", bufs=1) as wpool,
                tc.tile_pool(name="xsp", bufs=4) as xsp,
                tc.tile_pool(name="psq", bufs=6, space="PSUM") as psq,
            ):
                wq_ts = []
                wkv_ts = []
                for k in range(KT):
                    wq_t = wpool.tile([128, 512], F32R, name=f"wq{k}")
                    wkv_t = wpool.tile([128, 128], F32R, name=f"wkv{k}")
                    nc.sync.dma_start(wq_t[:], wq_d[128 * k : 128 * (k + 1), :])
                    nc.sync.dma_start(wkv_t[:], wkv_d[128 * k : 128 * (k + 1), :])
                    wq_ts.append(wq_t)
                    wkv_ts.append(wkv_t)

                for sq in range(4):
                    c0 = 512 * sq
                    psums = [
                        psq.tile([128, 512], F32, name="psq_t", tag="psq_t")
                        for _ in range(5)
                    ]
                    xq = []
                    for k in range(KT):
                        x_t = xsp.tile([128, 512], F32R, name="xq_t")
                        nc.sync.dma_start(
                            x_t[:], xT_d[128 * k : 128 * (k + 1), c0 : c0 + 512]
                        )
                        xq.append(x_t)
                    for k in range(KT):
                        for mt in range(4):
                            nc.tensor.matmul(
                                psums[mt][:],
                                wq_ts[k][:, 128 * mt : 128 * (mt + 1)],
                                xq[k][:],
                                start=(k == 0),
                                stop=(k == KT - 1),
                            )
                        nc.tensor.matmul(
                            psums[4][:],
                            wkv_ts[k][:],
                            xq[k][:],
                            start=(k == 0),
                            stop=(k == KT - 1),
                        )
                    # RoPE epilogues: q m-tiles (2 heads each)
                    for mt in range(4):
                        ps = psums[mt]
                        t_all = workp.tile([128, 512], F32, tag="ra")
                        nc.scalar.copy(t_all[:], ps[:])
                        t_shuf = workp.tile([128, 512], F32, tag="rb")
                        nc.vector.stream_shuffle(t_shuf[:], t_all[:], SHUF_MASK)
                        t_cos = workp.tile([128, 512], F32, tag="rc")
                        nc.vector.tensor_mul(t_cos[:], t_all[:], cos_t[:, c0 : c0 + 512])
                        t_sin = workp.tile([128, 512], F32, tag="rd")
                        nc.vector.tensor_mul(t_sin[:], t_shuf[:], sin_t[:, c0 : c0 + 512])
                        nc.vector.tensor_add(
                            qTm[mt][:, c0 : c0 + 512], t_cos[:], t_sin[:]
                        )
                    # kv epilogue: k rope (rows 0:64) + v copy (rows 64:128)
                    ps = psums[4]
                    t_allk = workp.tile([128, 512], F32, tag="ra", name="t_allk")
                    nc.scalar.copy(t_allk[0:64, :], ps[0:64, :])
                    t_shufk = workp.tile([128, 512], F32, tag="rb", name="t_shufk")
                    nc.vector.stream_shuffle(t_shufk[0:64, :], t_allk[0:64, :], SHUF_MASK)
                    t_cosk = workp.tile([128, 512], F32, tag="rc", name="t_cosk")
                    nc.vector.tensor_mul(
                        t_cosk[0:64, :], t_allk[0:64, :], cos_t[0:64, c0 : c0 + 512]
                    )
                    t_sink = workp.tile([128, 512], F32, tag="rd", name="t_sink")
                    nc.vector.tensor_mul(
                        t_sink[0:64, :], t_shufk[0:64, :], sin_t[0:64, c0 : c0 + 512]
                    )
                    nc.vector.tensor_add(
                        kT2[0:64, c0 : c0 + 512], t_cosk[0:64, :], t_sink[0:64, :]
                    )
                    nc.vector.tensor_add(
                        kT2[64:128, c0 : c0 + 512], t_cosk[0:64, :], t_sink[0:64, :]
                    )
                    nc.vector.tensor_copy(vT[:, c0 : c0 + 512], ps[64:128, :])

                if debug:
                    dq = workp.tile([128, 512], F32, tag="ra", name="dq")
                    for sq4 in range(4):
                        nc.vector.tensor_copy(dq[:], qTm[0][:, 512*sq4:512*(sq4+1)])
                        nc.sync.dma_start(dbg_qT_d[:, 512*sq4:512*(sq4+1)], dq[:])
                        dk = workp.tile([128, 512], F32, tag="rb", name="dk")
                        nc.vector.tensor_copy(dk[:], kT2[:, 512*sq4:512*(sq4+1)])
                        nc.sync.dma_start(dbg_kT_d[:, 512*sq4:512*(sq4+1)], dk[:])
                # v transposes -> v_ext natural layout + ones block
                for i in range(NQT):
                    tr = psq.tile([128, 64], BF16, name="vtr", tag="vtr", bufs=2)
                    nc.tensor.transpose(tr[:], vT[:, 128 * i : 128 * (i + 1)], id_t[:])
                    nc.vector.tensor_copy(v_ext[i][:, 0:64], tr[:])
                    nc.vector.memset(v_ext[i][:, 64:128], 1.0)

            if debug:
                dvx = workp.tile([128, 128], F32, tag="sm", name="dvx")
                nc.vector.tensor_copy(dvx[:], v_ext[2][:])
                nc.sync.dma_start(dbg_vx_d[:], dvx[:])
            # ------------- Phase 2: attention + O-projection, per seq tile ----
            with (
                tc.tile_pool(name="aoutp", bufs=1) as aoutp,
                tc.tile_pool(name="wosp", bufs=1) as wosp,
                tc.tile_pool(name="epool", bufs=18) as epool,
                tc.tile_pool(name="outsp", bufs=3) as outsp,
                tc.tile_pool(name="psS", bufs=2, space="PSUM") as psS,
                tc.tile_pool(name="psO", bufs=4, space="PSUM") as psO,
                tc.tile_pool(name="psP", bufs=2, space="PSUM") as psP,
            ):
                attn_oT = [aoutp.tile([128, S], F32R, name=f"aoT{t}") for t in range(4)]
                wo_sb = []
                for t in range(4):
                    w_t = wosp.tile([128, D], F32R, name=f"wo{t}")
                    nc.sync.dma_start(w_t[:], wo_d[128 * t : 128 * (t + 1), :])
                    wo_sb.append(w_t)

                e_prev = None

                for j in range(NQT):
                    W = 256 if j < 15 else 128
                    e_cur = []
                    for h in range(8):
                        rb = 64 * (h % 2)
                        pss = psS.tile([128, 256], F32, name="pss", tag="pss")
                        nc.tensor.matmul(
                            pss[:, 0:W],
                            kT2[rb : rb + 64, 128 * j : 128 * (j + 1)],
                            qTm[h // 2][rb : rb + 64, 128 * j : 128 * j + W],
                            start=True,
                            stop=True,
                        )
                        s_m = workp.tile([128, 256], F32, tag="sm")
                        nc.vector.tensor_add(
                            s_m[:, 0:W],
                            pss[:, 0:W],
                            ma0_t[:, 0:W] if j < 15 else ma1_t[:],
                        )
                        e_t = epool.tile([128, 256], BF16, tag="e")
                        nc.scalar.activation(e_t[:, 0:W], s_m[:, 0:W], Exp, scale=0.125)
                        e_cur.append(e_t)
                    # AV + denominators for qtile j: keys from tiles j-1 and j,
                    # contiguous 2-matmul accumulation group per head
                    po = [
                        psO.tile([128, 512], F32, name="po", tag="po")
                        for _ in range(2)
                    ]
                    for h in range(8):
                        g, hh = h // 4, h % 4
                        if j > 0:
                            nc.tensor.matmul(
                                po[g][:, 128 * hh : 128 * (hh + 1)],
                                v_ext[j - 1][:],
                                e_prev[h][:, 128:256],
                                start=True,
                                stop=False,
                            )
                        nc.tensor.matmul(
                            po[g][:, 128 * hh : 128 * (hh + 1)],
                            v_ext[j][:],
                            e_cur[h][:, 0:128],
                            start=(j == 0),
                            stop=True,
                        )
                    # normalize qtile j -> attn_oT
                    for h in range(8):
                        g, hh = h // 4, h % 4
                        pgo = po[g]
                        rec = workp.tile([64, 128], F32, tag="rec")
                        nc.vector.reciprocal(
                            rec[:], pgo[64:128, 128 * hh : 128 * (hh + 1)]
                        )
                        t, rb = h // 2, 64 * (h % 2)
                        nc.vector.tensor_mul(
                            attn_oT[t][rb : rb + 64, 128 * j : 128 * (j + 1)],
                            pgo[0:64, 128 * hh : 128 * (hh + 1)],
                            rec[:],
                        )
                    e_prev = e_cur
                    if debug:
                        dao = workp.tile([128, 128], F32, tag="sm", name="dao")
                        nc.vector.tensor_copy(dao[:], attn_oT[0][:, 128*j:128*(j+1)])
                        nc.sync.dma_start(dbg_ao_d[:, 128*j:128*(j+1)], dao[:])
                    # O-projection for seq tile j
                    for ch in range(6):
                        pp = psP.tile([128, OCH], F32, name="pp", tag="pp")
                        for t in range(4):
                            nc.tensor.matmul(
                                pp[:],
                                attn_oT[t][:, 128 * j : 128 * (j + 1)],
                                wo_sb[t][:, OCH * ch : OCH * (ch + 1)],
                                start=(t == 0),
                                stop=(t == 3),
                            )
                        osb = outsp.tile([128, OCH], F32, tag="osb")
                        nc.any.tensor_copy(osb[:], pp[:])
                        nc.sync.dma_start(
                            out_d[128 * j : 128 * (j + 1), OCH * ch : OCH * (ch + 1)],
                            osb[:],
                        )
    nc.compile()
    return nc


def _prep_inputs(x, rope_cache, wq_w, wq_b, wk_w, wk_b, wv_w, wv_b, wo_w):
    """Build the shared + per-core input maps."""
    xT = np.zeros((DP, S), dtype=np.float32)
    xT[0:D, :] = np.ascontiguousarray(x[0].T)
    xT[D, :] = 1.0  # bias row

    cos = np.asarray(rope_cache[:, 0, :], dtype=np.float32)  # [S, 64]
    sin = np.asarray(rope_cache[:, 1, :], dtype=np.float32)
    cosP = cos[:, PERM].T  # [64, S] permuted head-dim rows
    sinP = sin[:, PERM].T
    sign = np.where(PERM < 32, -1.0, 1.0).astype(np.float32)[:, None]
    sinPs = sinP * sign
    cosT = np.concatenate([cosP, cosP], axis=0).astype(np.float32)  # [128, S]
    sinTs = np.concatenate([sinPs, sinPs], axis=0).astype(np.float32)

    b_idx = np.arange(128)[:, None]
    a_idx = np.arange(256)[None, :]
    ma0 = np.where((b_idx <= a_idx) & (a_idx < b_idx + WINDOW), 0.0, -1e30).astype(
        np.float32
    )
    ma1 = np.where(b_idx <= a_idx[:, :128], 0.0, -1e30).astype(np.float32)
    import ml_dtypes

    id64 = np.eye(64, dtype=np.float32).astype(ml_dtypes.bfloat16)

    shared = dict(xT=xT, cosT=cosT, sinTs=sinTs, ma0=ma0, ma1=ma1, id64=id64)

    in_maps = []
    for c in range(N_CORES):
        # wq slice: q heads [8c, 8c+8), head-dim permuted, transposed, bias row
        wq_rows = []
        bq_rows = []
        for hh in range(8):
            g = 8 * c + hh
            wq_rows.append(wq_w[64 * g + PERM, :])  # [64, D]
            bq_rows.append(wq_b[64 * g + PERM])
        wq_slice = np.concatenate(wq_rows, axis=0)  # [512, D]
        bq_slice = np.concatenate(bq_rows, axis=0)  # [512]
        wq_t = np.zeros((DP, 512), dtype=np.float32)
        wq_t[0:D, :] = wq_slice.T
        wq_t[D, :] = bq_slice

        wk_slice = wk_w[64 * c + PERM, :]  # [64, D] permuted
        bk_slice = wk_b[64 * c + PERM]
        wv_slice = wv_w[64 * c : 64 * (c + 1), :]  # unpermuted
        bv_slice = wv_b[64 * c : 64 * (c + 1)]
        wkv_t = np.zeros((DP, 128), dtype=np.float32)
        wkv_t[0:D, 0:64] = wk_slice.T
        wkv_t[0:D, 64:128] = wv_slice.T
        wkv_t[D, 0:64] = bk_slice
        wkv_t[D, 64:128] = bv_slice

        wo_t = np.ascontiguousarray(wo_w[:, 512 * c : 512 * (c + 1)].T).astype(
            np.float32
        )  # [512, D]

        in_maps.append(dict(shared, wq=wq_t, wkv=wkv_t, wo=wo_t))
    return in_maps


def kernel(
    x,
    rope_cache,
    wq_w,
    wq_b,
    wk_w,
    wk_b,
    wv_w,
    wv_b,
    wo_w,
    wo_b,
):
    global _COMPILED
    x = np.asarray(x, dtype=np.float32)
    rope_cache = np.asarray(rope_cache, dtype=np.float32)
    wq_w = np.asarray(wq_w, dtype=np.float32)
    wq_b = np.asarray(wq_b, dtype=np.float32)
    wk_w = np.asarray(wk_w, dtype=np.float32)
    wk_b = np.asarray(wk_b, dtype=np.float32)
    wv_w = np.asarray(wv_w, dtype=np.float32)
    wv_b = np.asarray(wv_b, dtype=np.float32)
    wo_w = np.asarray(wo_w, dtype=np.float32)
    wo_b = np.asarray(wo_b, dtype=np.float32)

    if _COMPILED is None:
        _COMPILED = _build()
    nc = _COMPILED

    in_maps = _prep_inputs(x, rope_cache, wq_w, wq_b, wk_w, wk_b, wv_w, wv_b, wo_w)
    res = run_bass_kernel_spmd(nc, in_maps, core_ids=list(range(N_CORES)), trace=False)
    out = np.zeros((S, D), dtype=np.float32)
    for c in range(N_CORES):
        out += res.results[c]["partial"]
    out += wo_b[None, :]
    return out.reshape(B, S, D).astype(np.float32)


# expose the compiled-module runner for test harnesses that want tracing
def run_traced(**inputs):
    global _COMPILED
    if _COMPILED is None:
        _COMPILED = _build()
    in_maps = _prep_inputs(
        np.asarray(inputs["x"], np.float32),
        np.asarray(inputs["rope_cache"], np.float32),
        np.asarray(inputs["wq_w"], np.float32),
        np.asarray(inputs["wq_b"], np.float32),
        np.asarray(inputs["wk_w"], np.float32),
        np.asarray(inputs["wk_b"], np.float32),
        np.asarray(inputs["wv_w"], np.float32),
        np.asarray(inputs["wv_b"], np.float32),
        np.asarray(inputs["wo_w"], np.float32),
    )
    res = run_bass_kernel_spmd(
        _COMPILED, in_maps, core_ids=list(range(N_CORES)), trace=True
    )
    out = np.zeros((S, D), dtype=np.float32)
    for c in range(N_CORES):
        out += res.results[c]["partial"]
    out += np.asarray(inputs["wo_b"], np.float32)[None, :]
    return out.reshape(B, S, D).astype(np.float32), res



# revision 19
# speedup vs baseline: 1.1107x; 1.1107x over previous
"""Sliding-window GQA attention block (RoPE + QKV proj + SDPA + O proj) on 8
Trainium2 NeuronCores, head-sharded (1 kv-head group = 8 q-heads per core).

Contract: kernel(**inputs) takes the FULL unsharded inputs from
setup_inputs() and returns the FULL [1, 2048, 2880] output.

Per-core plan (core c owns q-heads [8c, 8c+8), kv-head c), all matmul
operands bf16:
  - QKV projections stream 512-wide fp32-accumulated bf16 matmuls; RoPE in
    the PSUM epilogue (head-dim pre-permuted so rotate-half = partition-pair
    stream_shuffle). q lands in qT_all [64, 8*S] (heads along free dim in
    order [0,2,4,6,1,3,5,7]), k in kT [64, S], v transposed to natural
    v_ext [seq, 64v + 64ones] tiles.
  - Attention per 128-query tile j: scores for 4 heads at a time via ONE
    [64,128]x[64,4x128] matmul against key tiles j-1 / j (512-wide), additive
    band mask (DVE/GpSimd), exp on ACT (scale=1/8 folded) -> bf16 e tiles.
    AV: lhsT = v_ext (shared by all heads), rhs = e [k, (4h,128q)] -> out.T
    [64 v + 64 dup-denominators, (h,q)] in one PSUM tile per j.
  - Normalization: reciprocal_approx_fast on the denominator rows, then two
    strided [64,4,128] multiplies into attn_oT [128, 4*S] bf16.
  - O projection: 4x6 bf16 matmuls per tile, DMA'd to HBM straight from
    PSUM (fp32); host sums the 8 partials and adds wo_b.
"""
import sys

sys.path.insert(0, "/opt/trn_rl_repo")

import numpy as np

import concourse.bass as bass  # noqa: F401  (import keeps bass registered)
import concourse.tile as tile
from concourse import bacc, mybir
from concourse.bass_utils import run_bass_kernel_spmd

B, S, D = 1, 2048, 2880
H, KVH, HD = 64, 8, 64
WINDOW = 128
N_CORES = 8
DP = 2944  # padded contraction dim: 23 * 128 (2880 data + 1 ones row + pad)
KT = DP // 128  # 23 contraction tiles
NQT = S // 128  # 16 seq tiles
OCH = 480  # O-proj free chunk (6 * 480 = 2880)

F32 = mybir.dt.float32
BF16 = mybir.dt.bfloat16

# head order along qT_all free dim: g0 = pair-firsts, g1 = pair-seconds
HEAD_ORDER = [0, 2, 4, 6, 1, 3, 5, 7]

# head-dim permutation: pairs (t, t+32) adjacent -> rotate-half partner is
# the neighbouring partition (stream_shuffle mask i^1 within quadrants)
PERM = np.empty(HD, dtype=np.int64)
PERM[0::2] = np.arange(32)
PERM[1::2] = np.arange(32) + 32

_COMPILED = None


def _build(debug=False):
    nc = bacc.Bacc("TRN2", target_bir_lowering=False, debug=False)

    xT_d = nc.dram_tensor("xT", [DP, S], BF16, kind="ExternalInput").ap()
    wq_d = nc.dram_tensor("wq", [DP, 512], BF16, kind="ExternalInput").ap()
    wkv_d = nc.dram_tensor("wkv", [DP, 128], BF16, kind="ExternalInput").ap()
    wo_d = nc.dram_tensor("wo", [512, D], BF16, kind="ExternalInput").ap()
    cos_d = nc.dram_tensor("cosT", [128, S], F32, kind="ExternalInput").ap()
    sin_d = nc.dram_tensor("sinTs", [128, S], F32, kind="ExternalInput").ap()
    maB_d = nc.dram_tensor("maB", [128, 512], F32, kind="ExternalInput").ap()
    maA_d = nc.dram_tensor("maA", [128, 512], F32, kind="ExternalInput").ap()
    id_d = nc.dram_tensor("id64", [64, 64], BF16, kind="ExternalInput").ap()
    out_d = nc.dram_tensor("partial", [S, D], BF16, kind="ExternalOutput").ap()
    if debug:
        dbg_q_d = nc.dram_tensor("dbg_q", [64, 8 * S], BF16, kind="ExternalOutput").ap()
        dbg_k_d = nc.dram_tensor("dbg_k", [64, S], BF16, kind="ExternalOutput").ap()
        dbg_v_d = nc.dram_tensor("dbg_v", [64, S], BF16, kind="ExternalOutput").ap()
        dbg_ao_d = nc.dram_tensor(
            "dbg_ao", [128, 4 * S], BF16, kind="ExternalOutput"
        ).ap()
        dbg_e_d = nc.dram_tensor("dbg_e", [128, 1024], BF16, kind="ExternalOutput").ap()

    Exp = mybir.ActivationFunctionType.Exp
    SHUF_MASK = [i ^ 1 for i in range(32)]

    with tile.TileContext(nc) as tc:
        with (
            tc.tile_pool(name="constp", bufs=1) as constp,
            tc.tile_pool(name="qkvp", bufs=1) as qkvp,
            tc.tile_pool(name="vextp", bufs=1) as vextp,
            tc.tile_pool(name="workp", bufs=3) as workp,
        ):
            cos_t = constp.tile([128, S], F32)
            sin_t = constp.tile([128, S], F32)
            maB_t = constp.tile([128, 512], F32)
            maA_t = constp.tile([128, 512], F32)
            id_t = constp.tile([64, 64], BF16)

            # persistent activations
            qT_all = qkvp.tile([64, 8 * S], BF16, name="qT_all")
            kT = qkvp.tile([64, S], BF16, name="kT")
            vT = qkvp.tile([64, S], BF16, name="vT")
            attn_oT = qkvp.tile([128, 4 * S], BF16, name="attn_oT")
            wo_sb = [qkvp.tile([128, D], BF16, name=f"wo{t}") for t in range(4)]
            v_ext = [vextp.tile([128, 128], BF16, name=f"vx{i}") for i in range(NQT)]

            # ---------------- Phase 1: QKV projections + RoPE ----------------
            with (
                tc.tile_pool(name="wpool", bufs=1) as wpool,
                tc.tile_pool(name="xsp", bufs=28) as xsp,
                tc.tile_pool(name="psq", bufs=6, space="PSUM") as psq,
            ):
                # interleave weight + first-chunk-x DMAs in groups of 4 k-tiles
                # so the PE can start after the first group lands
                wq_ts = [
                    wpool.tile([128, 512], BF16, name=f"wq{k}") for k in range(KT)
                ]
                wkv_ts = [
                    wpool.tile([128, 128], BF16, name=f"wkv{k}") for k in range(KT)
                ]
                x0 = [xsp.tile([128, 512], BF16, name="xq_t", tag="xq") for _ in range(KT)]
                for g0 in range(0, KT, 4):
                    g1 = min(g0 + 4, KT)
                    for k in range(g0, g1):
                        nc.sync.dma_start(wq_ts[k][:], wq_d[128 * k : 128 * (k + 1), :])
                    for k in range(g0, g1):
                        nc.sync.dma_start(
                            wkv_ts[k][:], wkv_d[128 * k : 128 * (k + 1), :]
                        )
                    for k in range(g0, g1):
                        nc.sync.dma_start(x0[k][:], xT_d[128 * k : 128 * (k + 1), 0:512])
                # constants (small; after the first weight groups)
                nc.sync.dma_start(cos_t[:], cos_d[:])
                nc.sync.dma_start(sin_t[:], sin_d[:])
                nc.sync.dma_start(maB_t[:], maB_d[:])
                nc.sync.dma_start(maA_t[:], maA_d[:])
                nc.sync.dma_start(id_t[:], id_d[:])
                # wo prefetch (needed at phase 2 start)
                for t in range(4):
                    nc.sync.dma_start(wo_sb[t][:], wo_d[128 * t : 128 * (t + 1), :])

                for sq in range(4):
                    c0 = 512 * sq
                    if sq == 0:
                        xq = x0
                    else:
                        xq = [
                            xsp.tile([128, 512], BF16, name="xq_t", tag="xq")
                            for _ in range(KT)
                        ]
                        for k in range(KT):
                            nc.sync.dma_start(
                                xq[k][:], xT_d[128 * k : 128 * (k + 1), c0 : c0 + 512]
                            )
                    psums = [
                        psq.tile([128, 512], F32, name="psq_t", tag="psq_t")
                        for _ in range(5)
                    ]
                    for k in range(KT):
                        for mt in range(4):
                            nc.tensor.matmul(
                                psums[mt][:],
                                wq_ts[k][:, 128 * mt : 128 * (mt + 1)],
                                xq[k][:],
                                start=(k == 0),
                                stop=(k == KT - 1),
                            )
                        nc.tensor.matmul(
                            psums[4][:],
                            wkv_ts[k][:],
                            xq[k][:],
                            start=(k == 0),
                            stop=(k == KT - 1),
                        )
                    # RoPE epilogues: q m-tiles (2 heads each), all-bf16 DVE ops
                    for mt in range(4):
                        ps = psums[mt]
                        # fp32 through the shuffle (bf16 stream_shuffle is
                        # broken on TRN2 hardware); bf16 from the muls onward
                        t_all = workp.tile([128, 512], F32, tag="ra")
                        nc.scalar.copy(t_all[:], ps[:])
                        t_shuf = workp.tile([128, 512], F32, tag="rb")
                        nc.vector.stream_shuffle(t_shuf[:], t_all[:], SHUF_MASK)
                        t_cos = workp.tile([128, 512], BF16, tag="rc")
                        nc.vector.tensor_mul(t_cos[:], t_all[:], cos_t[:, c0 : c0 + 512])
                        t_sin = workp.tile([128, 512], BF16, tag="rd")
                        nc.vector.tensor_mul(t_sin[:], t_shuf[:], sin_t[:, c0 : c0 + 512])
                        # psum halves -> adjacent qT_all head blocks (host packs
                        # wq columns in HEAD_ORDER = [0,2,4,6,1,3,5,7])
                        b0 = 2 * mt * S
                        b1 = (2 * mt + 1) * S
                        nc.vector.tensor_add(
                            qT_all[:, b0 + c0 : b0 + c0 + 512],
                            t_cos[0:64, :],
                            t_sin[0:64, :],
                        )
                        nc.vector.tensor_add(
                            qT_all[:, b1 + c0 : b1 + c0 + 512],
                            t_cos[64:128, :],
                            t_sin[64:128, :],
                        )
                    # kv epilogue: k rope (rows 0:64) + v copy (rows 64:128)
                    ps = psums[4]
                    t_allk = workp.tile([128, 512], F32, tag="ra", name="t_allk")
                    nc.scalar.copy(t_allk[0:64, :], ps[0:64, :])
                    t_shufk = workp.tile([128, 512], F32, tag="rb", name="t_shufk")
                    nc.vector.stream_shuffle(t_shufk[0:64, :], t_allk[0:64, :], SHUF_MASK)
                    t_cosk = workp.tile([128, 512], BF16, tag="rc", name="t_cosk")
                    nc.vector.tensor_mul(
                        t_cosk[0:64, :], t_allk[0:64, :], cos_t[0:64, c0 : c0 + 512]
                    )
                    t_sink = workp.tile([128, 512], BF16, tag="rd", name="t_sink")
                    nc.vector.tensor_mul(
                        t_sink[0:64, :], t_shufk[0:64, :], sin_t[0:64, c0 : c0 + 512]
                    )
                    nc.vector.tensor_add(
                        kT[:, c0 : c0 + 512], t_cosk[0:64, :], t_sink[0:64, :]
                    )
                    nc.vector.tensor_copy(vT[:, c0 : c0 + 512], ps[64:128, :])
                    # v transposes for this chunk's 4 seq tiles
                    for i in range(4 * sq, 4 * sq + 4):
                        tr = psq.tile([128, 64], BF16, name="vtr", tag="vtr", bufs=2)
                        nc.tensor.transpose(tr[:], vT[:, 128 * i : 128 * (i + 1)], id_t[:])
                        nc.vector.tensor_copy(v_ext[i][:, 0:64], tr[:])
                        nc.gpsimd.memset(v_ext[i][:, 64:128], 1.0)

            # ------------- Phase 2: attention + O-projection, per seq tile ----
            with (
                tc.tile_pool(name="epool", bufs=6) as epool,
                tc.tile_pool(name="recp", bufs=2) as recp,
                tc.tile_pool(name="outsp", bufs=3) as outsp,
                tc.tile_pool(name="psS", bufs=3, space="PSUM") as psS,
                tc.tile_pool(name="psAV", bufs=1, space="PSUM") as psAV,
                tc.tile_pool(name="psP", bufs=3, space="PSUM") as psP,
            ):
                qv = qT_all.rearrange("p (h s) -> p h s", h=8)
                ao = attn_oT.rearrange("p (t s) -> p t s", t=4)

                def oproj(j):
                    for ch in range(6):
                        pp = psP.tile([128, OCH], F32, name="pp", tag="pp")
                        for t in range(4):
                            nc.tensor.matmul(
                                pp[:],
                                attn_oT[:, t * S + 128 * j : t * S + 128 * (j + 1)],
                                wo_sb[t][:, OCH * ch : OCH * (ch + 1)],
                                start=(t == 0),
                                stop=(t == 3),
                            )
                        osb = outsp.tile([128, OCH], BF16, tag="osb", name="osb")
                        nc.scalar.copy(osb[:], pp[:])
                        nc.sync.dma_start(
                            out_d[128 * j : 128 * (j + 1), OCH * ch : OCH * (ch + 1)],
                            osb[:],
                        )

                for j in range(NQT):
                    # scores + mask + exp for tile j (4 heads per matmul)
                    e_tiles = []  # [(ktile_idx, g, e)]
                    for g in range(2):
                        if j > 0:
                            sA = psS.tile([128, 512], F32, name="sA", tag="ps_s")
                            nc.tensor.matmul(
                                sA[:],
                                kT[:, 128 * (j - 1) : 128 * j],
                                qv[:, 4 * g : 4 * g + 4, 128 * j : 128 * (j + 1)],
                                start=True,
                                stop=True,
                            )
                            nc.vector.tensor_add(sA[:], sA[:], maA_t[:])
                            eA = epool.tile([128, 512], BF16, tag="e", name="eA")
                            nc.scalar.activation(eA[:], sA[:], Exp, scale=0.125)
                            e_tiles.append((j - 1, g, eA))
                        sB = psS.tile([128, 512], F32, name="sB", tag="ps_s")
                        nc.tensor.matmul(
                            sB[:],
                            kT[:, 128 * j : 128 * (j + 1)],
                            qv[:, 4 * g : 4 * g + 4, 128 * j : 128 * (j + 1)],
                            start=True,
                            stop=True,
                        )
                        nc.vector.tensor_add(sB[:], sB[:], maB_t[:])
                        eB = epool.tile([128, 512], BF16, tag="e", name="eB")
                        nc.scalar.activation(eB[:], sB[:], Exp, scale=0.125)
                        e_tiles.append((j, g, eB))
                    # O-projection for the previous tile fills the PE while the
                    # exp of tile j drains on the ACT engine
                    if j > 0:
                        oproj(j - 1)
                    # AV + denominators: psum = [64 v + 64 dup-denoms, (g,h,q)]
                    pav = psAV.tile([128, 1024], F32, name="pav", tag="pav")
                    for kt_i, g, e_t in sorted(e_tiles, key=lambda x: (x[1], x[0])):
                        nc.tensor.matmul(
                            pav[:, 512 * g : 512 * (g + 1)],
                            v_ext[kt_i][:],
                            e_t[:],
                            start=(kt_i == j - 1 or j == 0),
                            stop=(kt_i == j),
                        )
                    # normalize: 1/denominators once, two strided multiplies.
                    # (custom-DVE ops cannot read PSUM on HW: stage via ACT)
                    den = recp.tile([64, 1024], F32, name="den", tag="den")
                    nc.scalar.copy(den[:], pav[64:128, :])
                    rec = recp.tile([64, 1024], F32, name="rec", tag="rec")
                    nc.vector.reciprocal_approx_fast(rec[:], den[:])
                    for g in range(2):
                        nc.vector.tensor_mul(
                            ao[64 * g : 64 * (g + 1), :, 128 * j : 128 * (j + 1)],
                            pav[0:64, 512 * g : 512 * (g + 1)].rearrange(
                                "p (h q) -> p h q", h=4
                            ),
                            rec[:, 512 * g : 512 * (g + 1)].rearrange(
                                "p (h q) -> p h q", h=4
                            ),
                        )
                    if debug and j == 2:
                        # dump the two eB tiles for tile 2
                        ebs = [e for kt_i, g, e in e_tiles if kt_i == j]
                        nc.sync.dma_start(dbg_e_d[:, 0:512], ebs[0][:])
                        nc.sync.dma_start(dbg_e_d[:, 512:1024], ebs[1][:])
                oproj(NQT - 1)
                if debug:
                    nc.sync.dma_start(dbg_q_d[:], qT_all[:])
                    nc.sync.dma_start(dbg_k_d[:], kT[:])
                    nc.sync.dma_start(dbg_v_d[:], vT[:])
                    nc.sync.dma_start(dbg_ao_d[:], attn_oT[:])
    nc.compile()
    return nc


def _prep_inputs(x, rope_cache, wq_w, wq_b, wk_w, wk_b, wv_w, wv_b, wo_w):
    """Build the shared + per-core input maps."""
    import ml_dtypes

    bf16 = ml_dtypes.bfloat16

    xT = np.zeros((DP, S), dtype=np.float32)
    xT[0:D, :] = np.ascontiguousarray(x[0].T)
    xT[D, :] = 1.0  # bias row

    cos = np.asarray(rope_cache[:, 0, :], dtype=np.float32)  # [S, 64]
    sin = np.asarray(rope_cache[:, 1, :], dtype=np.float32)
    cosP = cos[:, PERM].T  # [64, S] permuted head-dim rows
    sinP = sin[:, PERM].T
    sign = np.where(PERM < 32, -1.0, 1.0).astype(np.float32)[:, None]
    sinPs = sinP * sign
    cosT = np.concatenate([cosP, cosP], axis=0)  # [128, S]
    sinTs = np.concatenate([sinPs, sinPs], axis=0)

    kk = np.arange(128)[:, None]
    qq = np.arange(128)[None, :]
    maB1 = np.where(kk <= qq, 0.0, -1e30).astype(np.float32)  # same-tile causal
    maA1 = np.where(qq < kk, 0.0, -1e30).astype(np.float32)  # prev-tile window
    maB = np.tile(maB1, (1, 4))
    maA = np.tile(maA1, (1, 4))

    id64 = np.eye(64, dtype=np.float32).astype(bf16)

    shared = dict(
        xT=xT.astype(bf16),
        cosT=cosT.astype(np.float32),
        sinTs=sinTs.astype(np.float32),
        maB=maB,
        maA=maA,
        id64=id64,
    )

    in_maps = []
    for c in range(N_CORES):
        # wq slice: q heads [8c, 8c+8) in block order HEAD_ORDER, head-dim
        # permuted, transposed, bias row
        wq_rows = []
        bq_rows = []
        for lh in HEAD_ORDER:
            g = 8 * c + lh
            wq_rows.append(wq_w[64 * g + PERM, :])  # [64, D]
            bq_rows.append(wq_b[64 * g + PERM])
        wq_slice = np.concatenate(wq_rows, axis=0)  # [512, D]
        bq_slice = np.concatenate(bq_rows, axis=0)  # [512]
        wq_t = np.zeros((DP, 512), dtype=np.float32)
        wq_t[0:D, :] = wq_slice.T
        wq_t[D, :] = bq_slice

        wk_slice = wk_w[64 * c + PERM, :]  # [64, D] permuted
        bk_slice = wk_b[64 * c + PERM]
        wv_slice = wv_w[64 * c : 64 * (c + 1), :]  # unpermuted
        bv_slice = wv_b[64 * c : 64 * (c + 1)]
        wkv_t = np.zeros((DP, 128), dtype=np.float32)
        wkv_t[0:D, 0:64] = wk_slice.T
        wkv_t[0:D, 64:128] = wv_slice.T
        wkv_t[D, 0:64] = bk_slice
        wkv_t[D, 64:128] = bv_slice

        wo_t = np.ascontiguousarray(wo_w[:, 512 * c : 512 * (c + 1)].T)  # [512, D]

        in_maps.append(
            dict(
                shared,
                wq=wq_t.astype(bf16),
                wkv=wkv_t.astype(bf16),
                wo=wo_t.astype(bf16),
            )
        )
    return in_maps


def _run(inputs, trace):
    global _COMPILED
    if _COMPILED is None:
        _COMPILED = _build()
    args = [
        np.asarray(inputs[k], dtype=np.float32)
        for k in (
            "x",
            "rope_cache",
            "wq_w",
            "wq_b",
            "wk_w",
            "wk_b",
            "wv_w",
            "wv_b",
            "wo_w",
        )
    ]
    in_maps = _prep_inputs(*args)
    res = run_bass_kernel_spmd(
        _COMPILED, in_maps, core_ids=list(range(N_CORES)), trace=trace
    )
    out = np.zeros((S, D), dtype=np.float32)
    for c in range(N_CORES):
        out += res.results[c]["partial"]
    out += np.asarray(inputs["wo_b"], np.float32)[None, :]
    return out.reshape(B, S, D).astype(np.float32), res


def kernel(**inputs):
    out, _ = _run(inputs, trace=False)
    return out


# expose the compiled-module runner for test harnesses that want tracing
def run_traced(**inputs):
    return _run(inputs, trace=True)


# revision 20
# speedup vs baseline: 1.1544x; 1.0394x over previous
"""Sliding-window GQA attention block (RoPE + QKV proj + SDPA + O proj) on 8
Trainium2 NeuronCores, head-sharded (1 kv-head group = 8 q-heads per core).

Contract: kernel(**inputs) takes the FULL unsharded inputs from
setup_inputs() and returns the FULL [1, 2048, 2880] output.

Per-core plan (core c owns q-heads [8c, 8c+8), kv-head c), all matmul
operands bf16:
  - QKV projections stream 512-wide bf16 matmuls (fp32 PSUM); RoPE in the
    PSUM epilogue (head-dim pre-permuted so rotate-half = partition-pair
    stream_shuffle, done in fp32 - bf16 shuffle is broken on HW). q lands in
    qT_all [64, 8*S] (heads along free dim, order [0,2,4,6,1,3,5,7]), k in
    kT [64, S], v transposed to v_ext [seq, 64v + 64ones] tiles.
  - Attention per 128-query tile j, 4 heads per matmul (all 8 q-heads share
    the core's kv head): one [128,1024] PSUM tile per head-group holds
    scores vs key tiles j-1 | j; one combined band-mask add (DVE), one exp
    (ACT, scale=1/8 folded) -> bf16 e tile. AV: lhsT = v_ext, rhs = e ->
    out.T [64 v + 64 dup-denominators, (h,q)].
  - Normalization: ACT copies denominators to SBUF (custom-DVE ops cannot
    read PSUM on HW), reciprocal_approx_fast, two strided multiplies into
    attn_oT [128, 4*S] bf16.
  - O projection per tile: 4x6 bf16 matmuls; PSUM chunks copied to one
    [128, 2880] bf16 row tile (copies split DVE/ACT), single DMA out.
  - DMAs are batched (one descriptor per x chunk / weight group) to keep
    the SP issue queue short; host sums the 8 bf16 partials and adds wo_b.
"""
import sys

sys.path.insert(0, "/opt/trn_rl_repo")

import numpy as np

import concourse.bass as bass  # noqa: F401  (import keeps bass registered)
import concourse.tile as tile
from concourse import bacc, mybir
from concourse.bass_utils import run_bass_kernel_spmd

B, S, D = 1, 2048, 2880
H, KVH, HD = 64, 8, 64
WINDOW = 128
N_CORES = 8
DP = 2944  # padded contraction dim: 23 * 128 (2880 data + 1 ones row + pad)
KT = DP // 128  # 23 contraction tiles
NQT = S // 128  # 16 seq tiles
OCH = 480  # O-proj free chunk (6 * 480 = 2880)
WQG = [6, 6, 6, 5]  # wq k-tile DMA groups

F32 = mybir.dt.float32
BF16 = mybir.dt.bfloat16

# head order along qT_all free dim: g0 = pair-firsts, g1 = pair-seconds
HEAD_ORDER = [0, 2, 4, 6, 1, 3, 5, 7]

# head-dim permutation: pairs (t, t+32) adjacent -> rotate-half partner is
# the neighbouring partition (stream_shuffle mask i^1 within quadrants)
PERM = np.empty(HD, dtype=np.int64)
PERM[0::2] = np.arange(32)
PERM[1::2] = np.arange(32) + 32

_COMPILED = None


def _build(debug=False):
    nc = bacc.Bacc("TRN2", target_bir_lowering=False, debug=False)

    xT_d = nc.dram_tensor("xT", [DP, S], BF16, kind="ExternalInput").ap()
    wq_d = nc.dram_tensor("wq", [DP, 512], BF16, kind="ExternalInput").ap()
    wkv_d = nc.dram_tensor("wkv", [DP, 128], BF16, kind="ExternalInput").ap()
    wo_d = nc.dram_tensor("wo", [512, D], BF16, kind="ExternalInput").ap()
    cos_d = nc.dram_tensor("cosT", [128, S], F32, kind="ExternalInput").ap()
    sin_d = nc.dram_tensor("sinTs", [128, S], F32, kind="ExternalInput").ap()
    maAB_d = nc.dram_tensor("maAB", [128, 1024], F32, kind="ExternalInput").ap()
    id_d = nc.dram_tensor("id64", [64, 64], BF16, kind="ExternalInput").ap()
    out_d = nc.dram_tensor("partial", [S, D], BF16, kind="ExternalOutput").ap()
    if debug:
        dbg_q_d = nc.dram_tensor("dbg_q", [64, 8 * S], BF16, kind="ExternalOutput").ap()
        dbg_k_d = nc.dram_tensor("dbg_k", [64, S], BF16, kind="ExternalOutput").ap()
        dbg_v_d = nc.dram_tensor("dbg_v", [64, S], BF16, kind="ExternalOutput").ap()
        dbg_ao_d = nc.dram_tensor(
            "dbg_ao", [128, 4 * S], BF16, kind="ExternalOutput"
        ).ap()
        dbg_e_d = nc.dram_tensor("dbg_e", [128, 2048], BF16, kind="ExternalOutput").ap()

    Exp = mybir.ActivationFunctionType.Exp
    SHUF_MASK = [i ^ 1 for i in range(32)]

    # DRAM views with the 128-partition dim explicit, for batched DMAs
    xT_v = xT_d.rearrange("(k p) s -> p k s", p=128)  # [128, 23, 2048]
    wq_v = wq_d.rearrange("(k p) m -> p k m", p=128)  # [128, 23, 512]
    wkv_v = wkv_d.rearrange("(k p) m -> p k m", p=128)  # [128, 23, 128]

    with tile.TileContext(nc) as tc:
        with (
            tc.tile_pool(name="constp", bufs=1) as constp,
            tc.tile_pool(name="qkvp", bufs=1) as qkvp,
            tc.tile_pool(name="vextp", bufs=1) as vextp,
            tc.tile_pool(name="workp", bufs=3) as workp,
        ):
            cos_t = constp.tile([128, S], F32)
            sin_t = constp.tile([128, S], F32)
            maAB_t = constp.tile([128, 1024], F32)
            id_t = constp.tile([64, 64], BF16)

            # persistent activations
            qT_all = qkvp.tile([64, 8 * S], BF16, name="qT_all")
            kT = qkvp.tile([64, S], BF16, name="kT")
            vT = qkvp.tile([64, S], BF16, name="vT")
            attn_oT = qkvp.tile([128, 4 * S], BF16, name="attn_oT")
            wo_sb = [qkvp.tile([128, D], BF16, name=f"wo{t}") for t in range(4)]
            v_ext = [vextp.tile([128, 128], BF16, name=f"vx{i}") for i in range(NQT)]

            # ---------------- Phase 1: QKV projections + RoPE ----------------
            with (
                tc.tile_pool(name="wpool", bufs=1) as wpool,
                tc.tile_pool(name="xsp", bufs=2) as xsp,
                tc.tile_pool(name="psq", bufs=6, space="PSUM") as psq,
            ):
                wq_g = [
                    wpool.tile([128, 512 * n], BF16, name=f"wqg{g}")
                    for g, n in enumerate(WQG)
                ]
                wkv_t = wpool.tile([128, 128 * KT], BF16, name="wkv_t")
                xcs = [
                    xsp.tile([128, 512 * KT], BF16, name="xc", tag="xc")
                    for _ in range(4)
                ]
                # interleaved weight/x DMA order so the PE starts early
                k0 = 0
                for g, n in enumerate(WQG):
                    nc.sync.dma_start(
                        wq_g[g].rearrange("p (k m) -> p k m", m=512),
                        wq_v[:, k0 : k0 + n, :],
                    )
                    if g == 0:
                        nc.sync.dma_start(
                            wkv_t.rearrange("p (k m) -> p k m", m=128), wkv_v
                        )
                    nc.sync.dma_start(
                        xcs[0].rearrange("p (k s) -> p k s", s=512)[:, k0 : k0 + n, :],
                        xT_v[:, k0 : k0 + n, 0:512],
                    )
                    k0 += n
                nc.sync.dma_start(cos_t[:], cos_d[:])
                nc.sync.dma_start(sin_t[:], sin_d[:])
                nc.sync.dma_start(maAB_t[:], maAB_d[:])
                nc.sync.dma_start(id_t[:], id_d[:])
                for t in range(4):
                    nc.sync.dma_start(wo_sb[t][:], wo_d[128 * t : 128 * (t + 1), :])
                for sq in range(1, 4):
                    nc.sync.dma_start(
                        xcs[sq].rearrange("p (k s) -> p k s", s=512),
                        xT_v[:, :, 512 * sq : 512 * (sq + 1)],
                    )

                def wq_slice(k, mt):
                    g, i = 0, k
                    for n in WQG:
                        if i < n:
                            break
                        g, i = g + 1, i - n
                    c = 512 * i + 128 * mt
                    return wq_g[g][:, c : c + 128]

                for sq in range(4):
                    c0 = 512 * sq
                    xq = xcs[sq]
                    psums = [
                        psq.tile([128, 512], F32, name="psq_t", tag="psq_t")
                        for _ in range(5)
                    ]
                    for k in range(KT):
                        xk = xq[:, 512 * k : 512 * (k + 1)]
                        for mt in range(4):
                            nc.tensor.matmul(
                                psums[mt][:],
                                wq_slice(k, mt),
                                xk,
                                start=(k == 0),
                                stop=(k == KT - 1),
                            )
                        nc.tensor.matmul(
                            psums[4][:],
                            wkv_t[:, 128 * k : 128 * (k + 1)],
                            xk,
                            start=(k == 0),
                            stop=(k == KT - 1),
                        )
                    # RoPE epilogues: q m-tiles (2 heads each)
                    for mt in range(4):
                        ps = psums[mt]
                        # fp32 through the shuffle (bf16 stream_shuffle is
                        # broken on TRN2 hardware); bf16 from the muls onward
                        t_all = workp.tile([128, 512], F32, tag="ra")
                        nc.scalar.copy(t_all[:], ps[:])
                        t_shuf = workp.tile([128, 512], F32, tag="rb")
                        nc.vector.stream_shuffle(t_shuf[:], t_all[:], SHUF_MASK)
                        t_cos = workp.tile([128, 512], BF16, tag="rc")
                        nc.vector.tensor_mul(t_cos[:], t_all[:], cos_t[:, c0 : c0 + 512])
                        t_sin = workp.tile([128, 512], BF16, tag="rd")
                        nc.vector.tensor_mul(t_sin[:], t_shuf[:], sin_t[:, c0 : c0 + 512])
                        # psum halves -> adjacent qT_all head blocks (host packs
                        # wq columns in HEAD_ORDER = [0,2,4,6,1,3,5,7])
                        b0 = 2 * mt * S
                        b1 = (2 * mt + 1) * S
                        nc.vector.tensor_add(
                            qT_all[:, b0 + c0 : b0 + c0 + 512],
                            t_cos[0:64, :],
                            t_sin[0:64, :],
                        )
                        nc.vector.tensor_add(
                            qT_all[:, b1 + c0 : b1 + c0 + 512],
                            t_cos[64:128, :],
                            t_sin[64:128, :],
                        )
                    # kv epilogue: k rope (rows 0:64) + v copy (rows 64:128)
                    ps = psums[4]
                    t_allk = workp.tile([128, 512], F32, tag="ra", name="t_allk")
                    nc.scalar.copy(t_allk[0:64, :], ps[0:64, :])
                    t_shufk = workp.tile([128, 512], F32, tag="rb", name="t_shufk")
                    nc.vector.stream_shuffle(t_shufk[0:64, :], t_allk[0:64, :], SHUF_MASK)
                    t_cosk = workp.tile([128, 512], BF16, tag="rc", name="t_cosk")
                    nc.vector.tensor_mul(
                        t_cosk[0:64, :], t_allk[0:64, :], cos_t[0:64, c0 : c0 + 512]
                    )
                    t_sink = workp.tile([128, 512], BF16, tag="rd", name="t_sink")
                    nc.vector.tensor_mul(
                        t_sink[0:64, :], t_shufk[0:64, :], sin_t[0:64, c0 : c0 + 512]
                    )
                    nc.vector.tensor_add(
                        kT[:, c0 : c0 + 512], t_cosk[0:64, :], t_sink[0:64, :]
                    )
                    nc.vector.tensor_copy(vT[:, c0 : c0 + 512], ps[64:128, :])
                    # v transposes for this chunk's 4 seq tiles
                    for i in range(4 * sq, 4 * sq + 4):
                        tr = psq.tile([128, 64], BF16, name="vtr", tag="vtr", bufs=2)
                        nc.tensor.transpose(tr[:], vT[:, 128 * i : 128 * (i + 1)], id_t[:])
                        nc.vector.tensor_copy(v_ext[i][:, 0:64], tr[:])
                        nc.gpsimd.memset(v_ext[i][:, 64:128], 1.0)

            # ------------- Phase 2: attention + O-projection, per seq tile ----
            with (
                tc.tile_pool(name="epool", bufs=4) as epool,
                tc.tile_pool(name="recp", bufs=2) as recp,
                tc.tile_pool(name="outsp", bufs=2) as outsp,
                tc.tile_pool(name="psS", bufs=2, space="PSUM") as psS,
                tc.tile_pool(name="psAV", bufs=1, space="PSUM") as psAV,
                tc.tile_pool(name="psP", bufs=2, space="PSUM") as psP,
            ):
                qv = qT_all.rearrange("p (h s) -> p h s", h=8)
                ao = attn_oT.rearrange("p (t s) -> p t s", t=4)

                def oproj(j):
                    out_row = outsp.tile([128, D], BF16, tag="orow", name="orow")
                    for ch in range(6):
                        pp = psP.tile([128, OCH], F32, name="pp", tag="pp")
                        for t in range(4):
                            nc.tensor.matmul(
                                pp[:],
                                attn_oT[:, t * S + 128 * j : t * S + 128 * (j + 1)],
                                wo_sb[t][:, OCH * ch : OCH * (ch + 1)],
                                start=(t == 0),
                                stop=(t == 3),
                            )
                        dst = out_row[:, OCH * ch : OCH * (ch + 1)]
                        if ch < 3:
                            nc.vector.tensor_copy(dst, pp[:])
                        else:
                            nc.scalar.copy(dst, pp[:])
                    nc.sync.dma_start(out_d[128 * j : 128 * (j + 1), :], out_row[:])

                for j in range(NQT):
                    # scores + mask + exp, 4 heads per matmul; per head-group
                    # one [128,1024] psum = [keys j-1 | keys j]
                    e_g = []
                    for g in range(2):
                        sc = psS.tile([128, 1024], F32, name="sc", tag="ps_s")
                        qslc = qv[:, 4 * g : 4 * g + 4, 128 * j : 128 * (j + 1)]
                        if j > 0:
                            nc.tensor.matmul(
                                sc[:, 0:512],
                                kT[:, 128 * (j - 1) : 128 * j],
                                qslc,
                                start=True,
                                stop=True,
                            )
                        nc.tensor.matmul(
                            sc[:, 512:1024],
                            kT[:, 128 * j : 128 * (j + 1)],
                            qslc,
                            start=True,
                            stop=True,
                        )
                        e_t = epool.tile([128, 1024], BF16, tag="e", name="e_t")
                        if j > 0:
                            nc.vector.tensor_add(sc[:], sc[:], maAB_t[:])
                            nc.scalar.activation(e_t[:], sc[:], Exp, scale=0.125)
                        else:
                            nc.vector.tensor_add(
                                sc[:, 512:1024], sc[:, 512:1024], maAB_t[:, 512:1024]
                            )
                            nc.scalar.activation(
                                e_t[:, 512:1024], sc[:, 512:1024], Exp, scale=0.125
                            )
                        e_g.append(e_t)
                    # O-projection for the previous tile fills the PE while the
                    # exp of tile j drains on ACT
                    if j > 0:
                        oproj(j - 1)
                    # AV + denominators: psum = [64 v + 64 dup-denoms, (g,h,q)]
                    pav = psAV.tile([128, 1024], F32, name="pav", tag="pav")
                    for g in range(2):
                        if j > 0:
                            nc.tensor.matmul(
                                pav[:, 512 * g : 512 * (g + 1)],
                                v_ext[j - 1][:],
                                e_g[g][:, 0:512],
                                start=True,
                                stop=False,
                            )
                        nc.tensor.matmul(
                            pav[:, 512 * g : 512 * (g + 1)],
                            v_ext[j][:],
                            e_g[g][:, 512:1024],
                            start=(j == 0),
                            stop=True,
                        )
                    # normalize: 1/denominators once, two strided multiplies.
                    # (custom-DVE ops cannot read PSUM on HW: stage via ACT)
                    den = recp.tile([64, 1024], F32, name="den", tag="den")
                    nc.scalar.copy(den[:], pav[64:128, :])
                    rec = recp.tile([64, 1024], F32, name="rec", tag="rec")
                    nc.vector.reciprocal_approx_fast(rec[:], den[:])
                    for g in range(2):
                        nc.vector.tensor_mul(
                            ao[64 * g : 64 * (g + 1), :, 128 * j : 128 * (j + 1)],
                            pav[0:64, 512 * g : 512 * (g + 1)].rearrange(
                                "p (h q) -> p h q", h=4
                            ),
                            rec[:, 512 * g : 512 * (g + 1)].rearrange(
                                "p (h q) -> p h q", h=4
                            ),
                        )
                    if debug and j == 2:
                        nc.sync.dma_start(dbg_e_d[:, 0:1024], e_g[0][:])
                        nc.sync.dma_start(dbg_e_d[:, 1024:2048], e_g[1][:])
                oproj(NQT - 1)
                if debug:
                    nc.sync.dma_start(dbg_q_d[:], qT_all[:])
                    nc.sync.dma_start(dbg_k_d[:], kT[:])
                    nc.sync.dma_start(dbg_v_d[:], vT[:])
                    nc.sync.dma_start(dbg_ao_d[:], attn_oT[:])
    nc.compile()
    return nc


def _prep_inputs(x, rope_cache, wq_w, wq_b, wk_w, wk_b, wv_w, wv_b, wo_w):
    """Build the shared + per-core input maps."""
    import ml_dtypes

    bf16 = ml_dtypes.bfloat16

    xT = np.zeros((DP, S), dtype=np.float32)
    xT[0:D, :] = np.ascontiguousarray(x[0].T)
    xT[D, :] = 1.0  # bias row

    cos = np.asarray(rope_cache[:, 0, :], dtype=np.float32)  # [S, 64]
    sin = np.asarray(rope_cache[:, 1, :], dtype=np.float32)
    cosP = cos[:, PERM].T  # [64, S] permuted head-dim rows
    sinP = sin[:, PERM].T
    sign = np.where(PERM < 32, -1.0, 1.0).astype(np.float32)[:, None]
    sinPs = sinP * sign
    cosT = np.concatenate([cosP, cosP], axis=0).astype(np.float32)  # [128, S]
    sinTs = np.concatenate([sinPs, sinPs], axis=0).astype(np.float32)

    kk = np.arange(128)[:, None]
    qq = np.arange(128)[None, :]
    maB1 = np.where(kk <= qq, 0.0, -1e30).astype(np.float32)  # same-tile causal
    maA1 = np.where(qq < kk, 0.0, -1e30).astype(np.float32)  # prev-tile window
    maAB = np.concatenate([np.tile(maA1, (1, 4)), np.tile(maB1, (1, 4))], axis=1)

    id64 = np.eye(64, dtype=np.float32).astype(bf16)

    shared = dict(
        xT=xT.astype(bf16),
        cosT=cosT,
        sinTs=sinTs,
        maAB=maAB,
        id64=id64,
    )

    in_maps = []
    for c in range(N_CORES):
        # wq slice: q heads [8c, 8c+8) in block order HEAD_ORDER, head-dim
        # permuted, transposed, bias row
        wq_rows = []
        bq_rows = []
        for lh in HEAD_ORDER:
            g = 8 * c + lh
            wq_rows.append(wq_w[64 * g + PERM, :])  # [64, D]
            bq_rows.append(wq_b[64 * g + PERM])
        wq_slice = np.concatenate(wq_rows, axis=0)  # [512, D]
        bq_slice = np.concatenate(bq_rows, axis=0)  # [512]
        wq_t = np.zeros((DP, 512), dtype=np.float32)
        wq_t[0:D, :] = wq_slice.T
        wq_t[D, :] = bq_slice

        wk_slice = wk_w[64 * c + PERM, :]  # [64, D] permuted
        bk_slice = wk_b[64 * c + PERM]
        wv_slice = wv_w[64 * c : 64 * (c + 1), :]  # unpermuted
        bv_slice = wv_b[64 * c : 64 * (c + 1)]
        wkv_t = np.zeros((DP, 128), dtype=np.float32)
        wkv_t[0:D, 0:64] = wk_slice.T
        wkv_t[0:D, 64:128] = wv_slice.T
        wkv_t[D, 0:64] = bk_slice
        wkv_t[D, 64:128] = bv_slice

        wo_t = np.ascontiguousarray(wo_w[:, 512 * c : 512 * (c + 1)].T)  # [512, D]

        in_maps.append(
            dict(
                shared,
                wq=wq_t.astype(bf16),
                wkv=wkv_t.astype(bf16),
                wo=wo_t.astype(bf16),
            )
        )
    return in_maps


def _run(inputs, trace):
    global _COMPILED
    if _COMPILED is None:
        _COMPILED = _build()
    args = [
        np.asarray(inputs[k], dtype=np.float32)
        for k in (
            "x",
            "rope_cache",
            "wq_w",
            "wq_b",
            "wk_w",
            "wk_b",
            "wv_w",
            "wv_b",
            "wo_w",
        )
    ]
    in_maps = _prep_inputs(*args)
    res = run_bass_kernel_spmd(
        _COMPILED, in_maps, core_ids=list(range(N_CORES)), trace=trace
    )
    out = np.zeros((S, D), dtype=np.float32)
    for c in range(N_CORES):
        out += res.results[c]["partial"]
    out += np.asarray(inputs["wo_b"], np.float32)[None, :]
    return out.reshape(B, S, D).astype(np.float32), res


def kernel(**inputs):
    out, _ = _run(inputs, trace=False)
    return out


# expose the compiled-module runner for test harnesses that want tracing
def run_traced(**inputs):
    return _run(inputs, trace=True)


# revision 22
# speedup vs baseline: 1.5025x; 1.3015x over previous
"""Sliding-window GQA attention block (RoPE + QKV proj + SDPA + O proj) on 8
Trainium2 NeuronCores, head-sharded (1 kv-head group = 8 q-heads per core).

Contract: kernel(**inputs) takes the FULL unsharded inputs from
setup_inputs() and returns the FULL [1, 2048, 2880] output.

Per-core plan (core c owns q-heads [8c, 8c+8), kv-head c), all matmul
operands bf16:
  - QKV projections stream 512-wide bf16 matmuls (fp32 PSUM); RoPE in the
    PSUM epilogue (head-dim pre-permuted so rotate-half = partition-pair
    stream_shuffle, done in fp32 - bf16 shuffle is broken on HW). q lands in
    per-chunk tiles qc[c] [64, 8*512] (heads along free dim, order
    [0,2,4,6,1,3,5,7]), k in kT [64, S], v transposed to v_ext
    [seq, 64v + 64ones] tiles. x and wq stream via batched DMAs.
  - Attention per 128-query tile j, 4 heads per matmul (all 8 q-heads share
    the core's kv head): one [128,1024] PSUM tile per head-group holds
    scores vs key tiles j-1 | j; combined band-mask add (DVE), one exp
    (ACT, scale=1/8 folded) -> bf16 e tile. AV: lhsT = v_ext, rhs = e ->
    out.T [64 v + 64 dup-denominators, (h,q)]; denominators staged to SBUF
    (custom-DVE cannot read PSUM on HW), reciprocal_approx_fast, strided
    multiplies into per-tile aoj [128, 4*128] bf16.
  - Deep software pipeline: iteration j runs scores_j, AV_{j-1},
    oproj_{j-2} so no engine waits on a same-iteration producer.
  - O projection: 4x6 bf16 matmuls; PSUM chunks copied (ACT) into one
    [128, 2880] bf16 row tile, single DMA out per tile.
  - Host sums the 8 bf16 partials and adds wo_b.
"""
import sys

sys.path.insert(0, "/opt/trn_rl_repo")

import numpy as np

import concourse.bass as bass  # noqa: F401  (import keeps bass registered)
import concourse.tile as tile
from concourse import bacc, mybir
from concourse.bass_utils import run_bass_kernel_spmd

B, S, D = 1, 2048, 2880
H, KVH, HD = 64, 8, 64
WINDOW = 128
N_CORES = 8
DP = 2944  # padded contraction dim: 23 * 128 (2880 data + 1 ones row + pad)
KT = DP // 128  # 23 contraction tiles
NQT = S // 128  # 16 seq tiles
OCH = 480  # O-proj free chunk (6 * 480 = 2880)
WQG = [2, 7, 7, 7]  # wq k-tile DMA groups (small first group -> early PE start)
XH = [12, 11]  # x chunk half-tile k splits

F32 = mybir.dt.float32
BF16 = mybir.dt.bfloat16

# head order along q free dim: g0 = pair-firsts, g1 = pair-seconds
HEAD_ORDER = [0, 2, 4, 6, 1, 3, 5, 7]

# head-dim permutation: pairs (t, t+32) adjacent -> rotate-half partner is
# the neighbouring partition (stream_shuffle mask i^1 within quadrants)
PERM = np.empty(HD, dtype=np.int64)
PERM[0::2] = np.arange(32)
PERM[1::2] = np.arange(32) + 32

_COMPILED = None


def _build(debug=False):
    nc = bacc.Bacc("TRN2", target_bir_lowering=False, debug=False)

    xT_d = nc.dram_tensor("xT", [DP, S], BF16, kind="ExternalInput").ap()
    wq_d = nc.dram_tensor("wq", [DP, 512], BF16, kind="ExternalInput").ap()
    wkv_d = nc.dram_tensor("wkv", [DP, 128], BF16, kind="ExternalInput").ap()
    wo_d = nc.dram_tensor("wo", [512, D], BF16, kind="ExternalInput").ap()
    cos_d = nc.dram_tensor("cosT", [128, S], F32, kind="ExternalInput").ap()
    sin_d = nc.dram_tensor("sinTs", [128, S], F32, kind="ExternalInput").ap()
    maAB_d = nc.dram_tensor("maAB", [128, 1024], F32, kind="ExternalInput").ap()
    id_d = nc.dram_tensor("id64", [64, 64], BF16, kind="ExternalInput").ap()
    out_d = nc.dram_tensor("partial", [S, D], BF16, kind="ExternalOutput").ap()
    if debug:
        dbg_q_d = nc.dram_tensor("dbg_q", [64, 8 * S], BF16, kind="ExternalOutput").ap()
        dbg_k_d = nc.dram_tensor("dbg_k", [64, S], BF16, kind="ExternalOutput").ap()
        dbg_v_d = nc.dram_tensor("dbg_v", [64, S], BF16, kind="ExternalOutput").ap()
        dbg_e_d = nc.dram_tensor("dbg_e", [128, 2048], BF16, kind="ExternalOutput").ap()

    Exp = mybir.ActivationFunctionType.Exp
    SHUF_MASK = [i ^ 1 for i in range(32)]

    # DRAM views with the 128-partition dim explicit, for batched DMAs
    xT_v = xT_d.rearrange("(k p) s -> p k s", p=128)  # [128, 23, 2048]
    wq_v = wq_d.rearrange("(k p) m -> p k m", p=128)  # [128, 23, 512]
    wkv_v = wkv_d.rearrange("(k p) m -> p k m", p=128)  # [128, 23, 128]

    with tile.TileContext(nc) as tc:
        with (
            tc.tile_pool(name="constp", bufs=1) as constp,
            tc.tile_pool(name="qkvp", bufs=1) as qkvp,
            tc.tile_pool(name="vextp", bufs=1) as vextp,
            tc.tile_pool(name="workp", bufs=2) as workp,
        ):
            cos_t = constp.tile([128, S], F32)
            sin_t = constp.tile([128, S], F32)
            maAB_t = constp.tile([128, 1024], F32)
            id_t = constp.tile([64, 64], BF16)

            # persistent activations: q per chunk, k/v full
            qc = [qkvp.tile([64, 8 * 512], BF16, name=f"qc{c}") for c in range(4)]
            kT = qkvp.tile([64, S], BF16, name="kT")
            vT = qkvp.tile([64, S], BF16, name="vT")
            wo_sb = [qkvp.tile([128, D], BF16, name=f"wo{t}") for t in range(4)]
            v_ext = [vextp.tile([128, 128], BF16, name=f"vx{i}") for i in range(NQT)]

            # ---------------- Phase 1: QKV projections + RoPE ----------------
            with (
                tc.tile_pool(name="wpool", bufs=1) as wpool,
                tc.tile_pool(name="xsp", bufs=4) as xsp,
                tc.tile_pool(name="psq", bufs=6, space="PSUM") as psq,
            ):
                wq_g = [
                    wpool.tile([128, 512 * n], BF16, name=f"wqg{g}")
                    for g, n in enumerate(WQG)
                ]
                wkv_t = wpool.tile([128, 128 * KT], BF16, name="wkv_t")
                # x chunks as half-tiles (k 0:12 | 12:23), 4-buf rotation so the
                # next chunk's first half streams while the current one computes
                xh = [
                    [
                        xsp.tile([128, 512 * n], BF16, name="xh", tag="xh")
                        for n in XH
                    ]
                    for _ in range(4)
                ]

                def dma_x(sq, h, k0, k1):
                    base = 0 if h == 0 else XH[0]
                    nc.sync.dma_start(
                        xh[sq][h].rearrange("p (k s) -> p k s", s=512)[
                            :, k0 - base : k1 - base, :
                        ],
                        xT_v[:, k0:k1, 512 * sq : 512 * (sq + 1)],
                    )

                # startup order: small wq group + wkv + first x slice first
                k0 = 0
                x0_parts = [[(0, 2)], [(2, 9)], [(9, 12), (12, 16)], [(16, 23)]]
                for g, n in enumerate(WQG):
                    nc.sync.dma_start(
                        wq_g[g].rearrange("p (k m) -> p k m", m=512),
                        wq_v[:, k0 : k0 + n, :],
                    )
                    if g == 0:
                        nc.sync.dma_start(
                            wkv_t.rearrange("p (k m) -> p k m", m=128), wkv_v
                        )
                    for a, b in x0_parts[g]:
                        dma_x(0, 0 if a < XH[0] else 1, a, b)
                    k0 += n
                nc.sync.dma_start(cos_t[:], cos_d[:])
                nc.sync.dma_start(sin_t[:], sin_d[:])
                nc.sync.dma_start(maAB_t[:], maAB_d[:])
                nc.sync.dma_start(id_t[:], id_d[:])
                for t in range(4):
                    nc.sync.dma_start(wo_sb[t][:], wo_d[128 * t : 128 * (t + 1), :])
                for sq in range(1, 4):
                    dma_x(sq, 0, 0, XH[0])
                    dma_x(sq, 1, XH[0], KT)

                def wq_slice(k, mt):
                    g, i = 0, k
                    for n in WQG:
                        if i < n:
                            break
                        g, i = g + 1, i - n
                    c = 512 * i + 128 * mt
                    return wq_g[g][:, c : c + 128]

                def x_slice(sq, k):
                    h = 0 if k < XH[0] else 1
                    i = k if h == 0 else k - XH[0]
                    return xh[sq][h][:, 512 * i : 512 * (i + 1)]

                for sq in range(4):
                    c0 = 512 * sq
                    psums = [
                        psq.tile([128, 512], F32, name="psq_t", tag="psq_t")
                        for _ in range(5)
                    ]
                    for k in range(KT):
                        xk = x_slice(sq, k)
                        for mt in range(4):
                            nc.tensor.matmul(
                                psums[mt][:],
                                wq_slice(k, mt),
                                xk,
                                start=(k == 0),
                                stop=(k == KT - 1),
                            )
                        nc.tensor.matmul(
                            psums[4][:],
                            wkv_t[:, 128 * k : 128 * (k + 1)],
                            xk,
                            start=(k == 0),
                            stop=(k == KT - 1),
                        )
                    # RoPE epilogues: q m-tiles (2 heads each)
                    for mt in range(4):
                        ps = psums[mt]
                        # fp32 through the shuffle (bf16 stream_shuffle is
                        # broken on TRN2 hardware); bf16 from the muls onward
                        t_all = workp.tile([128, 512], F32, tag="ra")
                        nc.scalar.copy(t_all[:], ps[:])
                        t_shuf = workp.tile([128, 512], F32, tag="rb")
                        nc.vector.stream_shuffle(t_shuf[:], t_all[:], SHUF_MASK)
                        t_cos = workp.tile([128, 512], BF16, tag="rc")
                        nc.vector.tensor_mul(t_cos[:], t_all[:], cos_t[:, c0 : c0 + 512])
                        t_sin = workp.tile([128, 512], BF16, tag="rd")
                        nc.vector.tensor_mul(t_sin[:], t_shuf[:], sin_t[:, c0 : c0 + 512])
                        # psum halves -> adjacent head blocks of this chunk's q
                        # tile (host packs wq columns in HEAD_ORDER)
                        b0 = 512 * (2 * mt)
                        b1 = 512 * (2 * mt + 1)
                        nc.vector.tensor_add(
                            qc[sq][:, b0 : b0 + 512], t_cos[0:64, :], t_sin[0:64, :]
                        )
                        nc.vector.tensor_add(
                            qc[sq][:, b1 : b1 + 512], t_cos[64:128, :], t_sin[64:128, :]
                        )
                    # kv epilogue: k rope (rows 0:64) + v copy (rows 64:128)
                    ps = psums[4]
                    t_allk = workp.tile([128, 512], F32, tag="ra", name="t_allk")
                    nc.scalar.copy(t_allk[0:64, :], ps[0:64, :])
                    t_shufk = workp.tile([128, 512], F32, tag="rb", name="t_shufk")
                    nc.vector.stream_shuffle(t_shufk[0:64, :], t_allk[0:64, :], SHUF_MASK)
                    t_cosk = workp.tile([128, 512], BF16, tag="rc", name="t_cosk")
                    nc.vector.tensor_mul(
                        t_cosk[0:64, :], t_allk[0:64, :], cos_t[0:64, c0 : c0 + 512]
                    )
                    t_sink = workp.tile([128, 512], BF16, tag="rd", name="t_sink")
                    nc.vector.tensor_mul(
                        t_sink[0:64, :], t_shufk[0:64, :], sin_t[0:64, c0 : c0 + 512]
                    )
                    nc.vector.tensor_add(
                        kT[:, c0 : c0 + 512], t_cosk[0:64, :], t_sink[0:64, :]
                    )
                    nc.vector.tensor_copy(vT[:, c0 : c0 + 512], ps[64:128, :])
                    # v transposes for this chunk's 4 seq tiles
                    for i in range(4 * sq, 4 * sq + 4):
                        tr = psq.tile([128, 64], BF16, name="vtr", tag="vtr", bufs=2)
                        nc.tensor.transpose(tr[:], vT[:, 128 * i : 128 * (i + 1)], id_t[:])
                        nc.vector.tensor_copy(v_ext[i][:, 0:64], tr[:])
                        nc.gpsimd.memset(v_ext[i][:, 64:128], 1.0)

            # ------------- Phase 2: attention + O-projection, pipelined -------
            with (
                tc.tile_pool(name="epool", bufs=6) as epool,
                tc.tile_pool(name="recp", bufs=2) as recp,
                tc.tile_pool(name="aop", bufs=4) as aop,
                tc.tile_pool(name="outsp", bufs=2) as outsp,
                tc.tile_pool(name="psS", bufs=2, space="PSUM") as psS,
                tc.tile_pool(name="psAV", bufs=1, space="PSUM") as psAV,
                tc.tile_pool(name="psP", bufs=2, space="PSUM") as psP,
            ):
                e_hist = {}
                ao_hist = {}

                def scores(j):
                    """Scores + mask + exp for tile j -> e_hist[j]."""
                    cqc = qc[j // 4].rearrange("p (b s) -> p b s", s=512)
                    jo = 128 * (j % 4)
                    e_g = []
                    for g in range(2):
                        sc = psS.tile([128, 1024], F32, name="sc", tag="ps_s")
                        qslc = cqc[:, 4 * g : 4 * g + 4, jo : jo + 128]
                        if j > 0:
                            nc.tensor.matmul(
                                sc[:, 0:512],
                                kT[:, 128 * (j - 1) : 128 * j],
                                qslc,
                                start=True,
                                stop=True,
                            )
                        nc.tensor.matmul(
                            sc[:, 512:1024],
                            kT[:, 128 * j : 128 * (j + 1)],
                            qslc,
                            start=True,
                            stop=True,
                        )
                        e_t = epool.tile([128, 1024], BF16, tag="e", name="e_t")
                        if j > 0:
                            nc.vector.tensor_add(sc[:], sc[:], maAB_t[:])
                            nc.scalar.activation(e_t[:], sc[:], Exp, scale=0.125)
                        else:
                            nc.vector.tensor_add(
                                sc[:, 512:1024], sc[:, 512:1024], maAB_t[:, 512:1024]
                            )
                            nc.scalar.activation(
                                e_t[:, 512:1024], sc[:, 512:1024], Exp, scale=0.125
                            )
                        e_g.append(e_t)
                    e_hist[j] = e_g

                def av_norm(j):
                    """AV + denominators + normalize for tile j -> ao_hist[j]."""
                    e_g = e_hist.pop(j)
                    pav = psAV.tile([128, 1024], F32, name="pav", tag="pav")
                    for g in range(2):
                        if j > 0:
                            nc.tensor.matmul(
                                pav[:, 512 * g : 512 * (g + 1)],
                                v_ext[j - 1][:],
                                e_g[g][:, 0:512],
                                start=True,
                                stop=False,
                            )
                        nc.tensor.matmul(
                            pav[:, 512 * g : 512 * (g + 1)],
                            v_ext[j][:],
                            e_g[g][:, 512:1024],
                            start=(j == 0),
                            stop=True,
                        )
                    # custom-DVE ops cannot read PSUM on HW: stage via ACT
                    den = recp.tile([64, 1024], F32, name="den", tag="den")
                    nc.scalar.copy(den[:], pav[64:128, :])
                    rec = recp.tile([64, 1024], F32, name="rec", tag="rec")
                    nc.vector.reciprocal_approx_fast(rec[:], den[:])
                    aoj = aop.tile([128, 512], BF16, tag="ao", name="aoj")
                    aov = aoj.rearrange("p (t s) -> p t s", t=4)
                    for g in range(2):
                        nc.vector.tensor_mul(
                            aov[64 * g : 64 * (g + 1), :, :],
                            pav[0:64, 512 * g : 512 * (g + 1)].rearrange(
                                "p (h q) -> p h q", h=4
                            ),
                            rec[:, 512 * g : 512 * (g + 1)].rearrange(
                                "p (h q) -> p h q", h=4
                            ),
                        )
                    ao_hist[j] = aoj

                def oproj(j):
                    aoj = ao_hist.pop(j)
                    out_row = outsp.tile([128, D], BF16, tag="orow", name="orow")
                    for ch in range(6):
                        pp = psP.tile([128, OCH], F32, name="pp", tag="pp")
                        for t in range(4):
                            nc.tensor.matmul(
                                pp[:],
                                aoj[:, 128 * t : 128 * (t + 1)],
                                wo_sb[t][:, OCH * ch : OCH * (ch + 1)],
                                start=(t == 0),
                                stop=(t == 3),
                            )
                        nc.scalar.copy(out_row[:, OCH * ch : OCH * (ch + 1)], pp[:])
                    nc.sync.dma_start(out_d[128 * j : 128 * (j + 1), :], out_row[:])

                for j in range(NQT):
                    scores(j)
                    if j >= 1:
                        av_norm(j - 1)
                    if j >= 2:
                        oproj(j - 2)
                    if debug and j == 2:
                        nc.sync.dma_start(dbg_e_d[:, 0:1024], e_hist[2][0][:])
                        nc.sync.dma_start(dbg_e_d[:, 1024:2048], e_hist[2][1][:])
                av_norm(NQT - 1)
                oproj(NQT - 2)
                oproj(NQT - 1)
                if debug:
                    for c in range(4):
                        nc.sync.dma_start(
                            dbg_q_d[:, 4096 * c : 4096 * (c + 1)], qc[c][:]
                        )
                    nc.sync.dma_start(dbg_k_d[:], kT[:])
                    nc.sync.dma_start(dbg_v_d[:], vT[:])
    nc.compile()
    return nc


def _prep_inputs(x, rope_cache, wq_w, wq_b, wk_w, wk_b, wv_w, wv_b, wo_w):
    """Build the shared + per-core input maps."""
    import ml_dtypes

    bf16 = ml_dtypes.bfloat16

    xT = np.zeros((DP, S), dtype=np.float32)
    xT[0:D, :] = np.ascontiguousarray(x[0].T)
    xT[D, :] = 1.0  # bias row

    cos = np.asarray(rope_cache[:, 0, :], dtype=np.float32)  # [S, 64]
    sin = np.asarray(rope_cache[:, 1, :], dtype=np.float32)
    cosP = cos[:, PERM].T  # [64, S] permuted head-dim rows
    sinP = sin[:, PERM].T
    sign = np.where(PERM < 32, -1.0, 1.0).astype(np.float32)[:, None]
    sinPs = sinP * sign
    cosT = np.concatenate([cosP, cosP], axis=0).astype(np.float32)  # [128, S]
    sinTs = np.concatenate([sinPs, sinPs], axis=0).astype(np.float32)

    kk = np.arange(128)[:, None]
    qq = np.arange(128)[None, :]
    maB1 = np.where(kk <= qq, 0.0, -1e30).astype(np.float32)  # same-tile causal
    maA1 = np.where(qq < kk, 0.0, -1e30).astype(np.float32)  # prev-tile window
    maAB = np.concatenate([np.tile(maA1, (1, 4)), np.tile(maB1, (1, 4))], axis=1)

    id64 = np.eye(64, dtype=np.float32).astype(bf16)

    shared = dict(
        xT=xT.astype(bf16),
        cosT=cosT,
        sinTs=sinTs,
        maAB=maAB,
        id64=id64,
    )

    in_maps = []
    for c in range(N_CORES):
        # wq slice: q heads [8c, 8c+8) in block order HEAD_ORDER, head-dim
        # permuted, transposed, bias row
        wq_rows = []
        bq_rows = []
        for lh in HEAD_ORDER:
            g = 8 * c + lh
            wq_rows.append(wq_w[64 * g + PERM, :])  # [64, D]
            bq_rows.append(wq_b[64 * g + PERM])
        wq_slice = np.concatenate(wq_rows, axis=0)  # [512, D]
        bq_slice = np.concatenate(bq_rows, axis=0)  # [512]
        wq_t = np.zeros((DP, 512), dtype=np.float32)
        wq_t[0:D, :] = wq_slice.T
        wq_t[D, :] = bq_slice

        wk_slice = wk_w[64 * c + PERM, :]  # [64, D] permuted
        bk_slice = wk_b[64 * c + PERM]
        wv_slice = wv_w[64 * c : 64 * (c + 1), :]  # unpermuted
        bv_slice = wv_b[64 * c : 64 * (c + 1)]
        wkv_t = np.zeros((DP, 128), dtype=np.float32)
        wkv_t[0:D, 0:64] = wk_slice.T
        wkv_t[0:D, 64:128] = wv_slice.T
        wkv_t[D, 0:64] = bk_slice
        wkv_t[D, 64:128] = bv_slice

        wo_t = np.ascontiguousarray(wo_w[:, 512 * c : 512 * (c + 1)].T)  # [512, D]

        in_maps.append(
            dict(
                shared,
                wq=wq_t.astype(bf16),
                wkv=wkv_t.astype(bf16),
                wo=wo_t.astype(bf16),
            )
        )
    return in_maps


def _run(inputs, trace):
    global _COMPILED
    if _COMPILED is None:
        _COMPILED = _build()
    args = [
        np.asarray(inputs[k], dtype=np.float32)
        for k in (
            "x",
            "rope_cache",
            "wq_w",
            "wq_b",
            "wk_w",
            "wk_b",
            "wv_w",
            "wv_b",
            "wo_w",
        )
    ]
    in_maps = _prep_inputs(*args)
    res = run_bass_kernel_spmd(
        _COMPILED, in_maps, core_ids=list(range(N_CORES)), trace=trace
    )
    out = np.zeros((S, D), dtype=np.float32)
    for c in range(N_CORES):
        out += res.results[c]["partial"]
    out += np.asarray(inputs["wo_b"], np.float32)[None, :]
    return out.reshape(B, S, D).astype(np.float32), res


def kernel(**inputs):
    out, _ = _run(inputs, trace=False)
    return out


# expose the compiled-module runner for test harnesses that want tracing
def run_traced(**inputs):
    return _run(inputs, trace=True)
